# revision 15
# baseline (speedup 1.0000x reference)
"""Trainium2 Bass kernel for strict-causal (pixelSNAIL) attention.

Problem: B=8, H=W=64 (N=4096), Ck=64, Cv=128, fp32.
    out[b] = softmax(mask(q@k^T/sqrt(Ck))) @ v   with strictly-causal mask
    (pixel i attends only to j < i; row 0 gets all-zero output).

Sharding: data-parallel over batch - one batch per NeuronCore, 8 cores.

Per-core algorithm (v3: transposed-score layout, no P transposes, bf16):
  - DVE-convert q,k to bf16; PE-transpose -> qT,kT [64, 4096] bf16.
  - For each q-chunk of 512 rows, loop over k-tiles j (causal extent),
    two j per PSUM tile pair:
      S^T[128k, q..] = kT_j^T @ qT_chunk   (bf16 matmul, PSUM, exact extent)
      P^T = exp(0.125*S^T)  ScalarE, PSUM->SBUF bf16 (valid region only)
      diagonal k-tile: strict-causal zeroing of P^T via DVE mask multiply
      O[128q, 129] += P^T_{j,i}^T @ [V_j | 1]  (bf16 matmul per q-tile i,
         PSUM accumulate over j; col 128 accumulates the softmax rowsum)
  - Normalize on DVE: recip = 1/(rowsum+eps); o_chunk = O * recip;
    one output DMA per chunk. Output lands in [q, v] layout directly.
"""

import os
import sys

sys.path.insert(0, "/opt/trn_rl_repo")

import numpy as np

import concourse.bass as bass
import concourse.bacc as bacc
import concourse.mybir as mybir
import concourse.tile as tile
from concourse.bass_utils import run_bass_kernel_spmd
from concourse.masks import make_identity

F32 = mybir.dt.float32
BF16 = mybir.dt.bfloat16
FP16 = mybir.dt.float16

B, H, W, CK, CV = 8, 64, 64, 64, 128
N = H * W            # 4096
NT = N // 128        # 32 q-tiles / k-tiles
NCHUNK = N // 512    # 8 q-chunks
SCALE = 1.0 / np.sqrt(CK)


def build_kernel(repeats=1):
    nc = bacc.Bacc("TRN2", target_bir_lowering=False, debug=False, num_devices=8)

    q = nc.dram_tensor("q", [N, CK], F32, kind="ExternalInput").ap()
    k = nc.dram_tensor("k", [N, CK], F32, kind="ExternalInput").ap()
    v = nc.dram_tensor("v", [N, CV], F32, kind="ExternalInput").ap()
    o = nc.dram_tensor("o", [N, CV], F32, kind="ExternalOutput").ap()

    with tile.TileContext(nc) as tc:
        with (
            tc.tile_pool(name="const", bufs=1) as const_pool,
            tc.tile_pool(name="stage", bufs=1) as stage_pool,
            tc.tile_pool(name="qkT", bufs=1) as qkt_pool,
            tc.tile_pool(name="vsb", bufs=1) as v_pool,
            tc.tile_pool(name="p", bufs=3) as p_pool,
            tc.tile_pool(name="osb", bufs=2) as o_pool,
            tc.tile_pool(name="stats", bufs=8) as stats_pool,
            tc.tile_pool(name="ps_s", bufs=2, space="PSUM") as ps_s,
            tc.tile_pool(name="ps_o", bufs=2, space="PSUM") as ps_o,
        ):
            def emit_body():
                # ---- constants ----
                ident = const_pool.tile([128, 128], F32)
                make_identity(nc, ident[:])
                ident_bf = const_pool.tile([128, 128], FP16)
                nc.vector.tensor_copy(ident_bf[:], ident[:])
                # strict-causal keep-mask for diagonal tiles of P^T[k, q]:
                # 1.0 where k < q (partition < column), else 0.0
                mask_bf = const_pool.tile([128, 128], FP16)
                nc.gpsimd.memset(mask_bf[:], 1.0)
                nc.gpsimd.affine_select(
                    out=mask_bf[:],
                    in_=mask_bf[:],
                    compare_op=mybir.AluOpType.is_gt,  # keep 1 where q - k > 0
                    fill=0.0,
                    base=0,
                    pattern=[[1, 128]],
                    channel_multiplier=-1,
                )

                # ---- load q, k, v; convert to bf16; v_aug = [V | 1] ----
                q_stg = stage_pool.tile([128, NT, CK], F32, tag="q_stage")
                k_stg = stage_pool.tile([128, NT, CK], F32, tag="k_stage")
                vstg = stage_pool.tile([128, NT, CV], F32, tag="v_stage")
                q_bf = stage_pool.tile([128, NT, CK], FP16, tag="q_bf")
                k_bf = stage_pool.tile([128, NT, CK], FP16, tag="k_bf")
                v_aug = v_pool.tile([128, NT, CV + 1], FP16)

                nc.vector.memset(v_aug[:, :, CV], 1.0)

                q_r = q.rearrange("(t p) c -> p t c", p=128)
                k_r = k.rearrange("(t p) c -> p t c", p=128)
                v_r = v.rearrange("(t p) c -> p t c", p=128)
                for d in range(8):
                    sl = slice(4 * d, 4 * (d + 1))
                    nc.sync.dma_start(k_stg[:, sl, :], k_r[:, sl, :])
                    nc.sync.dma_start(q_stg[:, sl, :], q_r[:, sl, :])
                    nc.gpsimd.dma_start(vstg[:, sl, :], v_r[:, sl, :])
                    nc.vector.tensor_copy(k_bf[:, sl, :], k_stg[:, sl, :])
                    nc.vector.tensor_copy(q_bf[:, sl, :], q_stg[:, sl, :])
                    nc.vector.tensor_copy(v_aug[:, sl, :CV], vstg[:, sl, :])

                # ---- lazy PE transposes q,k -> qT,kT [64, N] bf16 ----
                qT = qkt_pool.tile([64, N], FP16, tag="qT")
                kT = qkt_pool.tile([64, N], FP16, tag="kT")

                def make_qkt(g, stg, dst):
                    def emit():
                        tp = ps_s.tile([64, 2048], FP16, tag="s", name="tp")
                        for u in range(4):
                            t = 4 * g + u
                            nc.tensor.transpose(
                                tp[:, u * 128 : (u + 1) * 128],
                                stg[:, t, :],
                                ident_bf[:],
                            )
                        nc.vector.tensor_copy(
                            dst[:, g * 512 : (g + 1) * 512], tp[:, :512]
                        )

                    return emit

                make_qkt(0, k_bf, kT)()
                make_qkt(0, q_bf, qT)()
                qk_pending = [
                    make_qkt(g, stg, dst)
                    for g in range(1, NT // 4)
                    for stg, dst in ((q_bf, qT), (k_bf, kT))
                ]
                qk_done = [0]

                def flush_qk(up_to_group):
                    while qk_done[0] < up_to_group and qk_pending:
                        qk_pending.pop(0)()
                        qk_pending.pop(0)()
                        qk_done[0] += 1

                # ---- main loop over q-chunks ----
                for c in range(NCHUNK):
                    flush_qk(min(c + 1, NT // 4 - 1))
                    njs = 4 * c + 4
                    o_ps = [
                        ps_o.tile([128, 2 * (CV + 1)], F32, tag="o01", name="o01"),
                        ps_o.tile([128, 2 * (CV + 1)], F32, tag="o23", name="o23"),
                    ]

                    carry_pv = [None]

                    def make_pv(c, p_t, ja, jb):
                        def emit():
                            for u, j in enumerate((ja, jb)):
                                base = 512 * u
                                t0 = max(0, j - 4 * c)
                                for i in range(t0, 4):
                                    # Two accumulation groups share each PSUM
                                    # bank. start=True pending-zeroes the WHOLE
                                    # 2KB bank, so only the first group (even i)
                                    # starts; the odd group's first write rides
                                    # the bank-wide pending-zero. Only the
                                    # last-finishing group (odd i) stops.
                                    nc.tensor.matmul(
                                        o_ps[i // 2][
                                            :,
                                            (i % 2) * (CV + 1) : (i % 2 + 1) * (CV + 1),
                                        ],
                                        p_t[:, base + 128 * i : base + 128 * (i + 1)],
                                        v_aug[:, j, :],
                                        start=(j == 0 and i % 2 == 0),
                                        stop=(i % 2 == 1 and j == 4 * c + i),
                                        skip_group_check=True,
                                    )

                        return emit

                    for u in range(njs // 2):
                        ja, jb = 2 * u, 2 * u + 1
                        s_ps = ps_s.tile([128, 1024], F32, tag="s", name="s_ps")
                        p_t = p_pool.tile([128, 1024], FP16, tag="p", name="p_t")
                        exts = []
                        for w, j in enumerate((ja, jb)):
                            base = 512 * w
                            t0 = max(0, j - 4 * c)
                            ext = 512 - 128 * t0     # valid q-columns
                            nc.tensor.matmul(
                                s_ps[:, base + 512 - ext : base + 512],
                                kT[:, 128 * j : 128 * (j + 1)],
                                qT[:, 512 * c + 512 - ext : 512 * (c + 1)],
                                start=True,
                                stop=True,
                            )
                            exts.append(ext)
                        ea, eb = exts
                        if ea == 512 and eb == 512:
                            nc.scalar.activation(
                                p_t[:, :1024],
                                s_ps[:, :1024],
                                mybir.ActivationFunctionType.Exp,
                                scale=SCALE,
                            )
                        else:
                            nc.scalar.activation(
                                p_t[:, 512 - ea : 512],
                                s_ps[:, 512 - ea : 512],
                                mybir.ActivationFunctionType.Exp,
                                scale=SCALE,
                            )
                            nc.scalar.activation(
                                p_t[:, 1024 - eb : 1024],
                                s_ps[:, 1024 - eb : 1024],
                                mybir.ActivationFunctionType.Exp,
                                scale=SCALE,
                            )
                        # strict-causal zeroing on the diagonal k-tiles
                        for w, j in enumerate((ja, jb)):
                            t0 = j - 4 * c
                            if 0 <= t0 <= 3:
                                sl = p_t[
                                    :, 512 * w + 128 * t0 : 512 * w + 128 * (t0 + 1)
                                ]
                                nc.vector.tensor_mul(sl, sl, mask_bf[:])
                        if carry_pv[0] is not None:
                            carry_pv[0]()
                        carry_pv[0] = make_pv(c, p_t, ja, jb)

                    carry_pv[0]()

                    # ---- normalize + store (one DMA per chunk) ----
                    o_ch = o_pool.tile([128, 4, CV], F32, tag="o_ch", name="o_ch")
                    for i in range(4):
                        sl = o_ps[i // 2][
                            :, (i % 2) * (CV + 1) : (i % 2 + 1) * (CV + 1)
                        ]
                        ssum = stats_pool.tile(
                            [128, 1], F32, tag=f"ss{i}", name="ssum"
                        )
                        nc.vector.tensor_scalar_add(
                            ssum[:], sl[:, CV : CV + 1], 1e-30
                        )
                        recip = stats_pool.tile(
                            [128, 1], F32, tag=f"rc{i}", name="recip"
                        )
                        nc.vector.reciprocal(recip[:], ssum[:])
                        nc.vector.tensor_scalar_mul(
                            o_ch[:, i, :], sl[:, :CV], recip[:]
                        )
                    nc.gpsimd.dma_start(
                        o[512 * c : 512 * (c + 1), :].rearrange(
                            "(t p) c -> p t c", p=128
                        ),
                        o_ch[:],
                    )

            if repeats > 1:
                with tc.For_i(0, repeats, 1):
                    emit_body()
            else:
                emit_body()

    nc.compile()
    return nc


_NC_CACHE = None


def kernel(**inputs: np.ndarray) -> np.ndarray:
    global _NC_CACHE
    if _NC_CACHE is None:
        _NC_CACHE = build_kernel()
    nc = _NC_CACHE

    query = np.ascontiguousarray(inputs["query"], dtype=np.float32)
    key = np.ascontiguousarray(inputs["key"], dtype=np.float32)
    value = np.ascontiguousarray(inputs["value"], dtype=np.float32)

    in_maps = [
        {
            "q": query[b].reshape(N, CK),
            "k": key[b].reshape(N, CK),
            "v": value[b].reshape(N, CV),
        }
        for b in range(B)
    ]
    res = run_bass_kernel_spmd(nc, in_maps, list(range(B)))
    out = np.stack([res.results[b]["o"] for b in range(B)], axis=0)
    return out.reshape(B, H, W, CV)


def run_traced(inputs_np):
    """Run with NTFF tracing, return HW exec time in ns (max over cores)."""
    global _NC_CACHE
    if _NC_CACHE is None:
        _NC_CACHE = build_kernel()
    nc = _NC_CACHE
    query = inputs_np["query"].reshape(B, N, CK)
    key = inputs_np["key"].reshape(B, N, CK)
    value = inputs_np["value"].reshape(B, N, CV)
    in_maps = [
        {"q": query[b], "k": key[b], "v": value[b]} for b in range(B)
    ]
    res = run_bass_kernel_spmd(nc, in_maps, list(range(B)), trace=True)
    return res.exec_time_ns


if __name__ == "__main__":
    rng = np.random.default_rng(0)
    qq = rng.standard_normal((B, H, W, CK), dtype=np.float32)
    kk = rng.standard_normal((B, H, W, CK), dtype=np.float32)
    vv = rng.standard_normal((B, H, W, CV), dtype=np.float32)
    out = kernel(query=qq, key=kk, value=vv)
    print("out", out.shape, out.dtype, np.abs(out).mean())


# revision 16
# speedup vs baseline: 2.3119x; 2.3119x over previous
"""Trainium2 Bass kernel for strict-causal (pixelSNAIL) attention.

Problem: B=8, H=W=64 (N=4096), Ck=64, Cv=128, fp32.
    out[b] = softmax(mask(q@k^T/sqrt(Ck))) @ v   with strictly-causal mask
    (pixel i attends only to j < i; row 0 gets all-zero output).

Sharding: data-parallel over batch - one batch per NeuronCore, 8 cores.

Per-core algorithm (v3: transposed-score layout, no P transposes, bf16):
  - DVE-convert q,k to bf16; PE-transpose -> qT,kT [64, 4096] bf16.
  - For each q-chunk of 512 rows, loop over k-tiles j (causal extent),
    two j per PSUM tile pair:
      S^T[128k, q..] = kT_j^T @ qT_chunk   (bf16 matmul, PSUM, exact extent)
      P^T = exp(0.125*S^T)  ScalarE, PSUM->SBUF bf16 (valid region only)
      diagonal k-tile: strict-causal zeroing of P^T via DVE mask multiply
      O[128q, 129] += P^T_{j,i}^T @ [V_j | 1]  (bf16 matmul per q-tile i,
         PSUM accumulate over j; col 128 accumulates the softmax rowsum)
  - Normalize on DVE: recip = 1/(rowsum+eps); o_chunk = O * recip;
    one output DMA per chunk. Output lands in [q, v] layout directly.
"""

import os
import sys

sys.path.insert(0, "/opt/trn_rl_repo")

import numpy as np

import concourse.bass as bass
import concourse.bacc as bacc
import concourse.mybir as mybir
import concourse.tile as tile
from concourse.bass_utils import run_bass_kernel_spmd
from concourse.masks import make_identity

F32 = mybir.dt.float32
BF16 = mybir.dt.bfloat16
FP16 = mybir.dt.float16

B, H, W, CK, CV = 8, 64, 64, 64, 128
N = H * W            # 4096
NT = N // 128        # 32 q-tiles / k-tiles
NCHUNK = N // 512    # 8 q-chunks
SCALE = 1.0 / np.sqrt(CK)


def build_kernel(repeats=1):
    nc = bacc.Bacc("TRN2", target_bir_lowering=False, debug=False, num_devices=8)

    q = nc.dram_tensor("q", [N, CK], F32, kind="ExternalInput").ap()
    k = nc.dram_tensor("k", [N, CK], F32, kind="ExternalInput").ap()
    v = nc.dram_tensor("v", [N, CV], F32, kind="ExternalInput").ap()
    o = nc.dram_tensor("o", [N, CV], F32, kind="ExternalOutput").ap()

    with tile.TileContext(nc) as tc:
        with (
            tc.tile_pool(name="const", bufs=1) as const_pool,
            tc.tile_pool(name="stage", bufs=1) as stage_pool,
            tc.tile_pool(name="qkT", bufs=1) as qkt_pool,
            tc.tile_pool(name="vsb", bufs=1) as v_pool,
            tc.tile_pool(name="p", bufs=3) as p_pool,
            tc.tile_pool(name="osb", bufs=2) as o_pool,
            tc.tile_pool(name="stats", bufs=8) as stats_pool,
            tc.tile_pool(name="ps_s", bufs=2, space="PSUM") as ps_s,
            tc.tile_pool(name="ps_o", bufs=2, space="PSUM") as ps_o,
        ):
            def emit_body():
                # ---- constants ----
                ident = const_pool.tile([128, 128], F32)
                make_identity(nc, ident[:])
                ident_bf = const_pool.tile([128, 128], FP16)
                nc.vector.tensor_copy(ident_bf[:], ident[:])
                # strict-causal keep-mask for diagonal tiles of P^T[k, q]:
                # 1.0 where k < q (partition < column), else 0.0
                mask_bf = const_pool.tile([128, 128], FP16)
                nc.gpsimd.memset(mask_bf[:], 1.0)
                nc.gpsimd.affine_select(
                    out=mask_bf[:],
                    in_=mask_bf[:],
                    compare_op=mybir.AluOpType.is_gt,  # keep 1 where q - k > 0
                    fill=0.0,
                    base=0,
                    pattern=[[1, 128]],
                    channel_multiplier=-1,
                )

                # ---- load q, k, v; convert to bf16; v_aug = [V | 1] ----
                q_stg = stage_pool.tile([128, NT, CK], F32, tag="q_stage")
                k_stg = stage_pool.tile([128, NT, CK], F32, tag="k_stage")
                vstg = stage_pool.tile([128, NT, CV], F32, tag="v_stage")
                q_bf = stage_pool.tile([128, NT, CK], FP16, tag="q_bf")
                k_bf = stage_pool.tile([128, NT, CK], FP16, tag="k_bf")
                v_aug = v_pool.tile([128, NT, CV + 1], FP16)

                nc.vector.memset(v_aug[:, :, CV], 1.0)

                q_r = q.rearrange("(t p) c -> p t c", p=128)
                k_r = k.rearrange("(t p) c -> p t c", p=128)
                v_r = v.rearrange("(t p) c -> p t c", p=128)
                for d in range(8):
                    sl = slice(4 * d, 4 * (d + 1))
                    nc.sync.dma_start(k_stg[:, sl, :], k_r[:, sl, :])
                    nc.sync.dma_start(q_stg[:, sl, :], q_r[:, sl, :])
                    nc.gpsimd.dma_start(vstg[:, sl, :], v_r[:, sl, :])
                    nc.vector.tensor_copy(k_bf[:, sl, :], k_stg[:, sl, :])
                    nc.vector.tensor_copy(q_bf[:, sl, :], q_stg[:, sl, :])
                    nc.vector.tensor_copy(v_aug[:, sl, :CV], vstg[:, sl, :])

                # ---- lazy PE transposes q,k -> qT,kT [64, N] bf16 ----
                qT = qkt_pool.tile([64, N], FP16, tag="qT")
                kT = qkt_pool.tile([64, N], FP16, tag="kT")

                def make_qkt(g, stg, dst):
                    def emit():
                        tp = ps_s.tile([64, 2048], FP16, tag="s", name="tp")
                        for u in range(4):
                            t = 4 * g + u
                            nc.tensor.transpose(
                                tp[:, u * 128 : (u + 1) * 128],
                                stg[:, t, :],
                                ident_bf[:],
                            )
                        nc.vector.tensor_copy(
                            dst[:, g * 512 : (g + 1) * 512], tp[:, :512]
                        )

                    return emit

                make_qkt(0, k_bf, kT)()
                make_qkt(0, q_bf, qT)()
                qk_pending = [
                    make_qkt(g, stg, dst)
                    for g in range(1, NT // 4)
                    for stg, dst in ((q_bf, qT), (k_bf, kT))
                ]
                qk_done = [0]

                def flush_qk(up_to_group):
                    while qk_done[0] < up_to_group and qk_pending:
                        qk_pending.pop(0)()
                        qk_pending.pop(0)()
                        qk_done[0] += 1

                # ---- main loop over q-chunks ----
                for c in range(NCHUNK):
                    flush_qk(min(c + 1, NT // 4 - 1))
                    njs = 4 * c + 4
                    o_ps = [
                        ps_o.tile([128, 2 * (CV + 1)], F32, tag="o01", name="o01"),
                        ps_o.tile([128, 2 * (CV + 1)], F32, tag="o23", name="o23"),
                    ]

                    carry_pv = [None]

                    def make_pv(c, p_t, ja, jb):
                        def emit():
                            for u, j in enumerate((ja, jb)):
                                t0 = max(0, j - 4 * c)
                                # ja right-aligned at 512; jb left-aligned at
                                # 512: slice for q-tile i starts at 128*i for
                                # ja, at 512 + 128*(i - t0) for jb.
                                for i in range(t0, 4):
                                    base = 128 * i if u == 0 else 512 + 128 * (i - t0)
                                    # Two accumulation groups share each PSUM
                                    # bank. start=True pending-zeroes the WHOLE
                                    # 2KB bank, so only the first group (even i)
                                    # starts; the odd group's first write rides
                                    # the bank-wide pending-zero. Only the
                                    # last-finishing group (odd i) stops.
                                    nc.tensor.matmul(
                                        o_ps[i // 2][
                                            :,
                                            (i % 2) * (CV + 1) : (i % 2 + 1) * (CV + 1),
                                        ],
                                        p_t[:, base : base + 128],
                                        v_aug[:, j, :],
                                        start=(j == 0 and i % 2 == 0),
                                        stop=(i % 2 == 1 and j == 4 * c + i),
                                        skip_group_check=True,
                                    )

                        return emit

                    for u in range(njs // 2):
                        ja, jb = 2 * u, 2 * u + 1
                        s_ps = ps_s.tile([128, 1024], F32, tag="s", name="s_ps")
                        p_t = p_pool.tile([128, 1024], FP16, tag="p", name="p_t")
                        # ja right-aligned (ends at col 512), jb
                        # left-aligned (starts at col 512): the pair's valid
                        # region [512-ea, 512+eb) is contiguous -> one exp.
                        exts = []
                        for w, j in enumerate((ja, jb)):
                            t0 = max(0, j - 4 * c)
                            ext = 512 - 128 * t0     # valid q-columns
                            lo = 512 - ext if w == 0 else 512
                            nc.tensor.matmul(
                                s_ps[:, lo : lo + ext],
                                kT[:, 128 * j : 128 * (j + 1)],
                                qT[:, 512 * c + 512 - ext : 512 * (c + 1)],
                                start=True,
                                stop=True,
                            )
                            exts.append(ext)
                        ea, eb = exts
                        nc.scalar.activation(
                            p_t[:, 512 - ea : 512 + eb],
                            s_ps[:, 512 - ea : 512 + eb],
                            mybir.ActivationFunctionType.Exp,
                            scale=SCALE,
                        )
                        # strict-causal zeroing on the diagonal k-tiles
                        for w, j in enumerate((ja, jb)):
                            t0 = j - 4 * c
                            if 0 <= t0 <= 3:
                                lo = 128 * t0 if w == 0 else 512
                                sl = p_t[:, lo : lo + 128]
                                nc.vector.tensor_mul(sl, sl, mask_bf[:])
                        if carry_pv[0] is not None:
                            carry_pv[0]()
                        carry_pv[0] = make_pv(c, p_t, ja, jb)

                    carry_pv[0]()

                    # ---- normalize + store (one DMA per chunk) ----
                    o_ch = o_pool.tile([128, 4, CV], F32, tag="o_ch", name="o_ch")
                    for i in range(4):
                        sl = o_ps[i // 2][
                            :, (i % 2) * (CV + 1) : (i % 2 + 1) * (CV + 1)
                        ]
                        ssum = stats_pool.tile(
                            [128, 1], F32, tag=f"ss{i}", name="ssum"
                        )
                        nc.vector.tensor_scalar_add(
                            ssum[:], sl[:, CV : CV + 1], 1e-30
                        )
                        recip = stats_pool.tile(
                            [128, 1], F32, tag=f"rc{i}", name="recip"
                        )
                        nc.vector.reciprocal(recip[:], ssum[:])
                        nc.vector.tensor_scalar_mul(
                            o_ch[:, i, :], sl[:, :CV], recip[:]
                        )
                    nc.gpsimd.dma_start(
                        o[512 * c : 512 * (c + 1), :].rearrange(
                            "(t p) c -> p t c", p=128
                        ),
                        o_ch[:],
                    )

            if repeats > 1:
                with tc.For_i(0, repeats, 1):
                    emit_body()
            else:
                emit_body()

    nc.compile()
    return nc


_NC_CACHE = None


def kernel(**inputs: np.ndarray) -> np.ndarray:
    global _NC_CACHE
    if _NC_CACHE is None:
        _NC_CACHE = build_kernel()
    nc = _NC_CACHE

    query = np.ascontiguousarray(inputs["query"], dtype=np.float32)
    key = np.ascontiguousarray(inputs["key"], dtype=np.float32)
    value = np.ascontiguousarray(inputs["value"], dtype=np.float32)

    in_maps = [
        {
            "q": query[b].reshape(N, CK),
            "k": key[b].reshape(N, CK),
            "v": value[b].reshape(N, CV),
        }
        for b in range(B)
    ]
    res = run_bass_kernel_spmd(nc, in_maps, list(range(B)))
    out = np.stack([res.results[b]["o"] for b in range(B)], axis=0)
    return out.reshape(B, H, W, CV)


def run_traced(inputs_np):
    """Run with NTFF tracing, return HW exec time in ns (max over cores)."""
    global _NC_CACHE
    if _NC_CACHE is None:
        _NC_CACHE = build_kernel()
    nc = _NC_CACHE
    query = inputs_np["query"].reshape(B, N, CK)
    key = inputs_np["key"].reshape(B, N, CK)
    value = inputs_np["value"].reshape(B, N, CV)
    in_maps = [
        {"q": query[b], "k": key[b], "v": value[b]} for b in range(B)
    ]
    res = run_bass_kernel_spmd(nc, in_maps, list(range(B)), trace=True)
    return res.exec_time_ns


if __name__ == "__main__":
    rng = np.random.default_rng(0)
    qq = rng.standard_normal((B, H, W, CK), dtype=np.float32)
    kk = rng.standard_normal((B, H, W, CK), dtype=np.float32)
    vv = rng.standard_normal((B, H, W, CV), dtype=np.float32)
    out = kernel(query=qq, key=kk, value=vv)
    print("out", out.shape, out.dtype, np.abs(out).mean())
